# revision 1
# baseline (speedup 1.0000x reference)
"""Int8Linear Trainium2 kernel (8 NeuronCores, batch-sharded).

Math (matches the jax reference):
  x_q   = round(x / s) + zp  (per-tensor affine, int8 range, no wrap for this data)
  xc    = x_q - zp = round(x / s)              <- zp cancels
  wc    = w_q - w_zp
  out   = (xc @ wc.T) * (s * w_scale) + bias

Device strategy per core c (core c owns batch c, M=2048 tokens):
  - x arrives pre-transposed [1024, 2048] f32 so the contraction dim (IN)
    is the SBUF partition dim; w arrives pre-transposed [1024, 4096] bf16
    (int8 values cast to bf16 exactly).
  - pass 1: per-k-tile max/min reduce -> partition all-reduce -> 8-core
    AllReduce(max) on [max, -min] -> s, 1/s, s*w_scale scalars on device.
  - pass 2: re-load x, q = rne(x * inv_s) via the +/-1.5*2^23 magic trick
    (hardware f32 add rounds to nearest-even, matching jnp.round), cast
    bf16 (exact: integers <= 256).
  - wc = w - w_zp in bf16 (exact).
  - GEMM: 16 m-tiles x 2 halves x 8 k x 4 o-chunks of 512; lhsT = xq
    slice (stationary), rhs = wc slice; PSUM tile [128, 2048] = 4 banks,
    2 tiles ping-pong.
  - epilogue: ACT copy psum*ss -> SBUF f32, DVE adds bias (broadcast to
    128 partitions once), DMA out.

All arithmetic on centered integer values is exact in bf16/f32, so the
only divergence vs the f32 reference is reciprocal-vs-divide rounding of
the quantization scale (~1e-3 relative on a tiny fraction of elements).
"""

import numpy as np
import ml_dtypes

B, S, IN, OUT = 8, 2048, 1024, 4096
NCORES = 8
KT = IN // 128          # 8 k-tiles
MT = S // 128           # 16 m-tiles
HALF = OUT // 2         # 2048
MAGIC = 12582912.0      # 1.5 * 2**23: forces f32 RNE at integer granularity

_cache = {}


def _build_program(reps=1, variant="full"):
    # variant:
    #   "reduce_only" - program A: per-core max / -min of x shard -> out "mm"
    #   "final"       - program B: quantize+GEMM, global max/-min from consts
    #   "full"        - single-dispatch version with on-device AllReduce
    #   "nocc"      - timing: skip the AllReduce (core-local max/min)
    #   "gemm_only" - timing: skip pass1/collective/quantize
    #   "noepi"     - timing: gemm_only, epilogue = raw psum copy
    import concourse.bass as bass
    import concourse.mybir as mybir
    import concourse.bacc as bacc
    import concourse.tile as tile
    from concourse import bass_isa

    f32 = mybir.dt.float32
    bf16 = mybir.dt.bfloat16
    Alu = mybir.AluOpType

    nc = bacc.Bacc(
        "TRN2",
        target_bir_lowering=False,
        debug=False,
        enable_asserts=True,
        num_devices=NCORES,
    )

    xt_d = nc.dram_tensor("xt", [IN, S], f32, kind="ExternalInput").ap()
    wt_d = nc.dram_tensor("wt", [IN, OUT], bf16, kind="ExternalInput").ap()
    bias_d = nc.dram_tensor("bias", [1, OUT], f32, kind="ExternalInput").ap()
    consts_d = nc.dram_tensor("consts", [1, 4], f32, kind="ExternalInput").ap()
    out_d = nc.dram_tensor("out", [S, OUT], f32, kind="ExternalOutput").ap()
    mm_d = nc.dram_tensor("mm", [1, 2], f32, kind="ExternalOutput").ap()

    with tile.TileContext(nc) as tc:
        with (
            tc.tile_pool(name="xin", bufs=3) as xin_pool,
            tc.tile_pool(name="tmp", bufs=2) as tmp_pool,
            tc.tile_pool(name="xq", bufs=KT) as xq_pool,
            tc.tile_pool(name="wq", bufs=KT) as wq_pool,
            tc.tile_pool(name="stats", bufs=1) as stats,
            tc.tile_pool(name="biasb", bufs=1) as biasb_pool,
            tc.tile_pool(name="osb", bufs=3) as osb_pool,
            tc.tile_pool(name="psum", bufs=2, space="PSUM") as psum_pool,
            tc.tile_pool(name="dram", bufs=1, space="DRAM") as dram_pool,
        ):
          for _rep in range(reps):
            # ---- constants / bias prep (no deps on x) ----
            bias_sb = stats.tile([1, OUT], f32, tag="bias_sb")
            nc.sync.dma_start(bias_sb[:], bias_d[:])
            consts_sb = stats.tile([1, 4], f32, tag="consts_sb")
            nc.sync.dma_start(consts_sb[:], consts_d[:])
            bias128 = biasb_pool.tile([128, OUT], f32, tag="bias128")
            wzp128 = stats.tile([128, 1], f32, tag="wzp128")
            inv128 = stats.tile([128, 1], f32, tag="inv128")
            ss128 = stats.tile([128, 1], f32, tag="ss128")
            if variant in ("full", "nocc", "final"):
                nc.gpsimd.partition_broadcast(bias128[:], bias_sb[:])
                nc.gpsimd.partition_broadcast(wzp128[:], consts_sb[0:1, 0:1])
            elif variant != "reduce_only":
                nc.vector.memset(bias128[:], 0.0)
                nc.vector.memset(wzp128[:], 0.0)
                nc.vector.memset(inv128[:], 1.0)
                nc.vector.memset(ss128[:], 1.0)

            if variant in ("full", "nocc", "reduce_only"):
                # ---- pass 1: max / min of this core's x shard ----
                mx_all = stats.tile([128, KT], f32, tag="mx_all")
                mn_all = stats.tile([128, KT], f32, tag="mn_all")
                for k in range(KT):
                    x_t = xin_pool.tile([128, S], f32, tag="xin")
                    nc.sync.dma_start(x_t[:], xt_d[k * 128:(k + 1) * 128, :])
                    nc.vector.tensor_reduce(
                        mx_all[:, k:k + 1], x_t[:], axis=mybir.AxisListType.X, op=Alu.max)
                    nc.vector.tensor_reduce(
                        mn_all[:, k:k + 1], x_t[:], axis=mybir.AxisListType.X, op=Alu.min)
                mx_p = stats.tile([128, 1], f32, tag="mx_p")
                mn_p = stats.tile([128, 1], f32, tag="mn_p")
                nc.vector.tensor_reduce(mx_p[:], mx_all[:], axis=mybir.AxisListType.X, op=Alu.max)
                nc.vector.tensor_reduce(mn_p[:], mn_all[:], axis=mybir.AxisListType.X, op=Alu.min)
                # negate min so a single AllReduce(max) handles both
                nmn_p = stats.tile([128, 1], f32, tag="nmn_p")
                nc.vector.tensor_scalar_mul(nmn_p[:], mn_p[:], -1.0)
                gmx = stats.tile([128, 1], f32, tag="gmx")
                gnm = stats.tile([128, 1], f32, tag="gnm")
                nc.gpsimd.partition_all_reduce(gmx[:], mx_p[:], channels=128,
                                               reduce_op=bass_isa.ReduceOp.max)
                nc.gpsimd.partition_all_reduce(gnm[:], nmn_p[:], channels=128,
                                               reduce_op=bass_isa.ReduceOp.max)

                if variant == "reduce_only":
                    mm_sb = stats.tile([1, 2], f32, tag="mm_sb")
                    nc.vector.tensor_copy(mm_sb[0:1, 0:1], gmx[0:1, :])
                    nc.vector.tensor_copy(mm_sb[0:1, 1:2], gnm[0:1, :])
                    nc.sync.dma_start(mm_d[:], mm_sb[:])
                if variant == "full":
                    cc_sb = stats.tile([1, 2], f32, tag="cc_sb")
                    nc.vector.tensor_copy(cc_sb[0:1, 0:1], gmx[0:1, :])
                    nc.vector.tensor_copy(cc_sb[0:1, 1:2], gnm[0:1, :])
                    cc_in = dram_pool.tile([1, 2], f32, tag="cc_in")
                    cc_out = dram_pool.tile([1, 2], f32, tag="cc_out")
                    nc.gpsimd.dma_start(cc_in[:], cc_sb[:])
                    nc.gpsimd.collective_compute(
                        "AllReduce",
                        Alu.max,
                        replica_groups=[list(range(NCORES))],
                        ins=[cc_in.opt()],
                        outs=[cc_out.opt()],
                    )
                    cc_res = stats.tile([1, 2], f32, tag="cc_res")
                    nc.gpsimd.dma_start(cc_res[:], cc_out[:])
                    gmax_ap, gnegmin_ap = cc_res[0:1, 0:1], cc_res[0:1, 1:2]
                else:
                    gmax_ap, gnegmin_ap = gmx[0:1, :], gnm[0:1, :]

            if variant == "final":
                gmax_ap, gnegmin_ap = consts_sb[0:1, 2:3], consts_sb[0:1, 3:4]
            if variant in ("full", "nocc", "final"):
                # ---- scalar chain: scale, inv_scale, ss = scale * w_scale ----
                d_t = stats.tile([1, 1], f32, tag="d_t")
                nc.vector.tensor_tensor(d_t[:], gmax_ap, gnegmin_ap, op=Alu.add)
                scale_t = stats.tile([1, 1], f32, tag="scale_t")
                nc.vector.tensor_scalar_mul(scale_t[:], d_t[:], 1.0 / 255.0)
                inv_t = stats.tile([1, 1], f32, tag="inv_t")
                nc.vector.reciprocal(inv_t[:], scale_t[:])
                ss_t = stats.tile([1, 1], f32, tag="ss_t")
                nc.vector.tensor_tensor(ss_t[:], scale_t[:], consts_sb[0:1, 1:2], op=Alu.mult)
                nc.gpsimd.partition_broadcast(inv128[:], inv_t[:])
                nc.gpsimd.partition_broadcast(ss128[:], ss_t[:])

            if variant == "reduce_only":
                continue
            # ---- w prep: wc = w - w_zp (bf16, exact) ----
            wq = []
            for k in range(KT):
                w_t = wq_pool.tile([128, OUT], bf16, tag="wq")
                nc.sync.dma_start(w_t[:], wt_d[k * 128:(k + 1) * 128, :])
                nc.vector.tensor_scalar(w_t[:], w_t[:], wzp128[:], None, op0=Alu.subtract)
                wq.append(w_t)

            # ---- pass 2: quantize x -> bf16 integers ----
            xq = []
            for k in range(KT):
                q_t = xq_pool.tile([128, S], bf16, tag="xq")
                if variant in ("full", "nocc", "final"):
                    x_t = xin_pool.tile([128, S], f32, tag="xin")
                    nc.sync.dma_start(x_t[:], xt_d[k * 128:(k + 1) * 128, :])
                    t1 = tmp_pool.tile([128, S], f32, tag="tmp")
                    nc.vector.tensor_scalar(t1[:], x_t[:], inv128[:], MAGIC,
                                            op0=Alu.mult, op1=Alu.add)
                    nc.vector.tensor_scalar(q_t[:], t1[:], MAGIC, None, op0=Alu.subtract)
                else:
                    nc.vector.memset(q_t[:], 1.0)
                xq.append(q_t)

            # ---- GEMM + epilogue ----
            Copy = mybir.ActivationFunctionType.Copy
            for m in range(MT):
                for h in range(2):
                    ps = psum_pool.tile([128, HALF], f32, tag="ps")
                    for k in range(KT):
                        lhsT = xq[k][:, m * 128:(m + 1) * 128]
                        for o in range(4):
                            col = h * HALF + o * 512
                            nc.tensor.matmul(
                                ps[:, o * 512:(o + 1) * 512],
                                lhsT,
                                wq[k][:, col:col + 512],
                                start=(k == 0),
                                stop=(k == KT - 1),
                            )
                    o_t = osb_pool.tile([128, HALF], f32, tag="osb")
                    if variant == "noepi":
                        nc.vector.tensor_copy(o_t[:], ps[:])
                    else:
                        nc.scalar.activation(o_t[:], ps[:], Copy, bias=0.0, scale=ss128[:])
                        nc.vector.tensor_tensor(
                            o_t[:], o_t[:], bias128[:, h * HALF:(h + 1) * HALF], op=Alu.add)
                    nc.sync.dma_start(
                        out_d[m * 128:(m + 1) * 128, h * HALF:(h + 1) * HALF], o_t[:])

    nc.compile()
    return nc


def kernel(x, w_q, w_scale, w_zp, bias, _bench=False):
    from concourse.bass_utils import run_bass_kernel_spmd

    x = np.asarray(x, dtype=np.float32)
    w_q = np.asarray(w_q)
    bias_f = np.asarray(bias, dtype=np.float32).reshape(1, OUT)

    # weight prep: transpose + exact cast of int8 values to bf16
    wt = np.ascontiguousarray(w_q.T.astype(np.float32)).astype(ml_dtypes.bfloat16)

    if "nc" not in _cache:
        _cache["nc"] = _build_program(variant="full")

    xts = [np.ascontiguousarray(x[c].T) for c in range(NCORES)]  # [IN, S] f32
    consts = np.array([[np.float32(w_zp), np.float32(w_scale), 0.0, 0.0]],
                      dtype=np.float32)
    in_maps = [{"xt": xts[c], "wt": wt, "bias": bias_f, "consts": consts}
               for c in range(NCORES)]
    res = run_bass_kernel_spmd(_cache["nc"], in_maps, list(range(NCORES)))
    out = np.stack([res.results[c]["out"] for c in range(NCORES)], axis=0)
    if _bench:
        return out, res
    return out

